# revision 11
# baseline (speedup 1.0000x reference)
"""Trainium2 kernel for nn_KernelizedAttention_14869176779022.

Math note: the reference computes
    out = (s * v) / s        with s = <phi_q, phi_k> > 0  (sums of exps)
so out == v == x @ Wv.T + bv exactly (up to one multiply/divide rounding).
The kernel therefore only computes the Wv linear layer; the bias add is
folded into the host-side gather (it's a per-element fp32 add on 32MB).

Sharding: data-parallel over the 8192 (B*S) positions - 1024 rows per core.

v2 layout/schedule (v1 measured 54906ns):
  * all DMA via the two HWDGE queues (SP + Activation sequencers) - v1's
    13 SWDGE dma_starts serialized ~9us of descriptor generation on GpSimd
    before the first input byte landed.
  * wv packed n-half-major, x packed m-major; loads issued in exactly the
    order the n-outer matmul loop consumes them, so the first matmul is
    gated on 0.75MB, not 4MB.
  * PE warm-up: dummy matmuls (no data deps) keep the Tensor engine busy
    from t0 so the DVFS p-state reaches 2.4GHz before the real matmuls
    start (cold PE runs at 1.2GHz for its first ~3us).
  * outputs in bf16 (rel-err stays ~3e-3), drained per PSUM bank-pair by
    DVE, stored as 16 2D HWDGE transfers alternating between both queues.
"""

import sys

if "/opt/trn_rl_repo" not in sys.path:
    sys.path.insert(0, "/opt/trn_rl_repo")

import numpy as np

B, S, E = 2, 4096, 1024
N_CORES = 8
ROWS = B * S            # 8192
R = ROWS // N_CORES     # 1024 rows per core
P = 128                 # partitions
KT = E // P             # 8 contraction tiles
MT = R // P             # 8 row tiles per core
NSZ = 512               # PSUM bank free size (fp32)
NT = E // NSZ           # 2 output-column halves
WARMUP_MM = 11          # dummy 512-row matmuls to ramp the PE p-state

# x chunk boundaries (in m-tiles): m0 rides the SP queue (faster first
# packet) right after the first wv chunk; the rest stream on the
# Activation queue.
X_CHUNKS = [(1, 3), (3, 5), (5, 7), (7, 8)]

_NC_CACHE = {}


def _build_nc(**bass_kwargs):
    import concourse.bass as bass
    import concourse.mybir as mybir
    from concourse import bacc
    from concourse.tile import TileContext

    f32 = mybir.dt.float32
    bf16 = mybir.dt.bfloat16
    nc = bacc.Bacc(None, target_bir_lowering=False, **bass_kwargs)

    # xb[p, (m*KT + k)*P + mm] = x_shard[m*P + mm, k*P + p]      (bf16)
    xb = nc.dram_tensor("xb", [P, MT * KT * P], bf16, kind="ExternalInput")
    # wv[p, (n*KT + k)*NSZ + j] = Wv[n*NSZ + j, k*P + p]         (bf16)
    wv = nc.dram_tensor("wv", [P, NT * KT * NSZ], bf16, kind="ExternalInput")
    out = nc.dram_tensor("out", [R, E], bf16, kind="ExternalOutput")

    with TileContext(nc) as tc:
        with (
            tc.tile_pool(name="consts", bufs=1) as consts,
            tc.tile_pool(name="opool", bufs=4) as opool,
            tc.tile_pool(name="ppool", bufs=3, space="PSUM") as ppool,
            tc.tile_pool(name="wpool", bufs=1, space="PSUM") as wpool,
        ):
            # --- PE warm-up: keep the Tensor engine busy from t0 so the
            # DVFS p-state has ramped when the real matmuls issue (the ramp
            # needs ~4.8us of continuous PE activity on HW). The scratch
            # PSUM bank is never read.
            wtile = consts.tile([P, NSZ], bf16, tag="wtile")
            nc.gpsimd.memset(wtile, 0.0)
            wscr = wpool.tile([P, NSZ], f32, tag="wscr")
            for _ in range(WARMUP_MM):
                nc.tensor.matmul(wscr, wtile[:, :P], wtile, start=True, stop=True)

            # --- input loads, in consumption order. The first-matmul gate
            # (wv n0/k0-3 + x m0) rides the SP queue, whose first packet
            # lands ~2us earlier than the Activation queue's.
            wv_sb = consts.tile([P, NT * KT * NSZ], bf16, tag="wv")
            x_sb = consts.tile([P, MT * KT * P], bf16, tag="x")
            kc2 = 2 * NSZ  # two k-tiles of one n-half
            nc.sync.dma_start(out=wv_sb[:, :kc2], in_=wv[:, :kc2])
            nc.sync.dma_start(out=x_sb[:, : KT * P], in_=xb[:, : KT * P])
            for c in range(1, 4):  # wv n0 k2-3, k4-5, k6-7
                nc.sync.dma_start(
                    out=wv_sb[:, c * kc2 : (c + 1) * kc2],
                    in_=wv[:, c * kc2 : (c + 1) * kc2],
                )
            for c in range(2):     # wv n1 in two 4-k-tile chunks
                nc.sync.dma_start(
                    out=wv_sb[:, (4 + 2 * c) * kc2 : (6 + 2 * c) * kc2],
                    in_=wv[:, (4 + 2 * c) * kc2 : (6 + 2 * c) * kc2],
                )
            for m0, m1 in X_CHUNKS:  # x m1..m7 on Activation queue
                nc.scalar.dma_start(
                    out=x_sb[:, m0 * KT * P : m1 * KT * P],
                    in_=xb[:, m0 * KT * P : m1 * KT * P],
                )

            # --- n-outer / m-pair loop: PSUM bank-pairs drained by DVE to
            # bf16, 2D stores alternating HWDGE queues. The final pair is
            # drained per-bank so the last store only trails the last
            # matmul by one 512-col DVE op.
            def drain_and_store(ps_ap, n, m, width, sq):
                om = opool.tile([P, width * NSZ], bf16, tag="om")
                nc.vector.tensor_scalar_add(om, ps_ap, 0.0)
                for mi in range(width):
                    dst = bass.AP(
                        tensor=out.tensor if hasattr(out, "tensor") else out,
                        offset=(m + mi) * P * E + n * NSZ,
                        ap=[[E, P], [1, NSZ]],
                    )
                    eng = nc.sync if sq == 0 else nc.scalar
                    sq ^= 1
                    eng.dma_start(out=dst, in_=om[:, mi * NSZ : (mi + 1) * NSZ])
                return sq

            sq = 0
            for n in range(NT):
                for mp in range(MT // 2):
                    ps = ppool.tile([P, 2 * NSZ], f32, tag="ps")
                    for mi in range(2):
                        m = mp * 2 + mi
                        for k in range(KT):
                            nc.tensor.matmul(
                                ps[:, mi * NSZ : (mi + 1) * NSZ],
                                x_sb[:, (m * KT + k) * P : (m * KT + k + 1) * P],
                                wv_sb[:, (n * KT + k) * NSZ : (n * KT + k + 1) * NSZ],
                                start=(k == 0),
                                stop=(k == KT - 1),
                            )
                        last = n == NT - 1 and mp == MT // 2 - 1
                        if last:  # drain each bank of the final pair ASAP
                            sq = drain_and_store(
                                ps[:, mi * NSZ : (mi + 1) * NSZ], n, m, 1, sq
                            )
                    if not last:
                        sq = drain_and_store(ps, n, mp * 2, 2, sq)
    nc.compile()
    return nc


def _get_nc():
    if "nc" not in _NC_CACHE:
        _NC_CACHE["nc"] = _build_nc()
    return _NC_CACHE["nc"]


def _prep_in_maps(x, Wv):
    import ml_dtypes

    bf16 = ml_dtypes.bfloat16
    x = np.ascontiguousarray(np.asarray(x, dtype=np.float32))
    Wv = np.asarray(Wv, dtype=np.float32)

    xf = x.reshape(ROWS, E)
    # wv[p, (n*KT+k)*NSZ + j] = Wv[n*NSZ + j, k*P + p]
    wvp = np.ascontiguousarray(
        Wv.reshape(NT, NSZ, KT, P)
        .transpose(3, 0, 2, 1)
        .reshape(P, NT * KT * NSZ)
        .astype(bf16)
    )

    in_maps = []
    for c in range(N_CORES):
        xs = xf[c * R : (c + 1) * R]                    # [R, E]
        # xb[p, (m*KT+k)*P+mm] = xs[m*P+mm, k*P+p]
        xbc = np.ascontiguousarray(
            xs.reshape(MT, P, KT, P)
            .transpose(3, 0, 2, 1)
            .reshape(P, MT * KT * P)
            .astype(bf16)
        )
        in_maps.append({"xb": xbc, "wv": wvp})
    return in_maps


def _install_ntff_hook():
    """This image's antenv lacks axon_hooks; recreate the bridge module so
    run_bass_kernel_spmd(trace=True) can reach the ctypes NTFF profiler."""
    import types

    if "antenv.axon_hooks" in sys.modules:
        return
    try:
        from trn_agent_boot.trn_boot import _ntff_profile_via_ctypes
    except ImportError:
        return
    hook = _ntff_profile_via_ctypes("/opt/axon/libaxon_pjrt.so")
    mod = types.ModuleType("antenv.axon_hooks")
    mod._hook = hook
    mod.get_axon_ntff_profile_hook = lambda: mod._hook
    mod.set_axon_ntff_profile_hook = lambda h: setattr(mod, "_hook", h)
    sys.modules["antenv.axon_hooks"] = mod


def _run(x, Wv, bv, trace=False):
    from concourse.bass_utils import run_bass_kernel_spmd

    if trace:
        _install_ntff_hook()
    nc = _get_nc()
    in_maps = _prep_in_maps(x, Wv)
    res = run_bass_kernel_spmd(
        nc, in_maps, core_ids=list(range(N_CORES)), trace=trace
    )
    out = np.concatenate(
        [np.asarray(res.results[c]["out"]) for c in range(N_CORES)], axis=0
    )
    out = out.astype(np.float32) + np.asarray(bv, dtype=np.float32)[None, :]
    return out.reshape(B, S, E), res


def kernel(x, Wq, bq, Wk, bk, Wv, bv, weights):
    out, _ = _run(x, Wv, bv, trace=False)
    return out


def kernel_traced(x, Wq, bq, Wk, bk, Wv, bv, weights):
    """Like kernel() but with NTFF profiling; returns (out, BassKernelResults)."""
    out, res = _run(x, Wv, bv, trace=True)
    return out, res


# revision 12
# speedup vs baseline: 1.0525x; 1.0525x over previous
"""Trainium2 kernel for nn_KernelizedAttention_14869176779022.

Math note: the reference computes
    out = (s * v) / s        with s = <phi_q, phi_k> > 0  (sums of exps)
so out == v == x @ Wv.T + bv exactly (up to one multiply/divide rounding).
The kernel therefore only computes the Wv linear layer; the bias add is
folded into the host-side gather (it's a per-element fp32 add on 32MB).

Sharding: data-parallel over the 8192 (B*S) positions - 1024 rows per core.

v2 layout/schedule (v1 measured 54906ns):
  * all DMA via the two HWDGE queues (SP + Activation sequencers) - v1's
    13 SWDGE dma_starts serialized ~9us of descriptor generation on GpSimd
    before the first input byte landed.
  * wv packed n-half-major, x packed m-major; loads issued in exactly the
    order the n-outer matmul loop consumes them, so the first matmul is
    gated on 0.75MB, not 4MB.
  * PE warm-up: dummy matmuls (no data deps) keep the Tensor engine busy
    from t0 so the DVFS p-state reaches 2.4GHz before the real matmuls
    start (cold PE runs at 1.2GHz for its first ~3us).
  * outputs in bf16 (rel-err stays ~3e-3), drained per PSUM bank-pair by
    DVE, stored as 16 2D HWDGE transfers alternating between both queues.
"""

import sys

if "/opt/trn_rl_repo" not in sys.path:
    sys.path.insert(0, "/opt/trn_rl_repo")

import numpy as np

B, S, E = 2, 4096, 1024
N_CORES = 8
ROWS = B * S            # 8192
R = ROWS // N_CORES     # 1024 rows per core
P = 128                 # partitions
KT = E // P             # 8 contraction tiles
MT = R // P             # 8 row tiles per core
NSZ = 512               # PSUM bank free size (fp32)
NT = E // NSZ           # 2 output-column halves
WARMUP_MM = 14          # dummy 512-row matmuls to ramp the PE p-state

# x chunk boundaries (in m-tiles): m0 rides the SP queue (faster first
# packet) right after the first wv chunk; the rest stream on the
# Activation queue.
X_CHUNKS = [(0, 2), (2, 4), (4, 6), (6, 8)]

_NC_CACHE = {}


def _build_nc(**bass_kwargs):
    import concourse.bass as bass
    import concourse.mybir as mybir
    from concourse import bacc
    from concourse.tile import TileContext

    f32 = mybir.dt.float32
    bf16 = mybir.dt.bfloat16
    nc = bacc.Bacc(None, target_bir_lowering=False, **bass_kwargs)

    # xb[p, (m*KT + k)*P + mm] = x_shard[m*P + mm, k*P + p]      (bf16)
    xb = nc.dram_tensor("xb", [P, MT * KT * P], bf16, kind="ExternalInput")
    # wv[p, (n*KT + k)*NSZ + j] = Wv[n*NSZ + j, k*P + p]         (bf16)
    wv = nc.dram_tensor("wv", [P, NT * KT * NSZ], bf16, kind="ExternalInput")
    out = nc.dram_tensor("out", [R, E], bf16, kind="ExternalOutput")

    with TileContext(nc) as tc:
        with (
            tc.tile_pool(name="consts", bufs=1) as consts,
            tc.tile_pool(name="opool", bufs=4) as opool,
            tc.tile_pool(name="ppool", bufs=3, space="PSUM") as ppool,
            tc.tile_pool(name="wpool", bufs=1, space="PSUM") as wpool,
        ):
            # --- PE warm-up: keep the Tensor engine busy from t0 so the
            # DVFS p-state has ramped when the real matmuls issue (the ramp
            # needs ~4.8us of continuous PE activity on HW). The scratch
            # PSUM bank is never read.
            wtile = consts.tile([P, NSZ], bf16, tag="wtile")
            nc.gpsimd.memset(wtile, 0.0)
            wscr = wpool.tile([P, NSZ], f32, tag="wscr")
            for _ in range(WARMUP_MM):
                nc.tensor.matmul(wscr, wtile[:, :P], wtile, start=True, stop=True)

            # --- input loads, in consumption order. The first-matmul gate
            # (wv n0/k0-3 + x m0) rides the SP queue, whose first packet
            # lands ~2us earlier than the Activation queue's.
            wv_sb = consts.tile([P, NT * KT * NSZ], bf16, tag="wv")
            x_sb = consts.tile([P, MT * KT * P], bf16, tag="x")
            kc = (KT // 2) * NSZ  # half of one n-half (4 k-tiles)
            for c in range(4):    # n0a, n0b, n1a, n1b on SP queue
                nc.sync.dma_start(
                    out=wv_sb[:, c * kc : (c + 1) * kc],
                    in_=wv[:, c * kc : (c + 1) * kc],
                )
            for m0, m1 in X_CHUNKS:  # x m1..m7 on Activation queue
                nc.scalar.dma_start(
                    out=x_sb[:, m0 * KT * P : m1 * KT * P],
                    in_=xb[:, m0 * KT * P : m1 * KT * P],
                )

            # --- n-outer / m-pair loop: PSUM bank-pairs drained by DVE to
            # bf16, 2D stores alternating HWDGE queues. The final pair is
            # drained per-bank so the last store only trails the last
            # matmul by one 512-col DVE op.
            def drain_and_store(ps_ap, n, m, width, sq):
                om = opool.tile([P, width * NSZ], bf16, tag="om")
                nc.vector.tensor_scalar_add(om, ps_ap, 0.0)
                for mi in range(width):
                    dst = bass.AP(
                        tensor=out.tensor if hasattr(out, "tensor") else out,
                        offset=(m + mi) * P * E + n * NSZ,
                        ap=[[E, P], [1, NSZ]],
                    )
                    eng = nc.sync if sq == 0 else nc.scalar
                    sq ^= 1
                    eng.dma_start(out=dst, in_=om[:, mi * NSZ : (mi + 1) * NSZ])
                return sq

            sq = 0
            for n in range(NT):
                for mp in range(MT // 2):
                    ps = ppool.tile([P, 2 * NSZ], f32, tag="ps")
                    for mi in range(2):
                        m = mp * 2 + mi
                        for k in range(KT):
                            nc.tensor.matmul(
                                ps[:, mi * NSZ : (mi + 1) * NSZ],
                                x_sb[:, (m * KT + k) * P : (m * KT + k + 1) * P],
                                wv_sb[:, (n * KT + k) * NSZ : (n * KT + k + 1) * NSZ],
                                start=(k == 0),
                                stop=(k == KT - 1),
                            )
                        last = n == NT - 1 and mp == MT // 2 - 1
                        if last:  # drain each bank of the final pair ASAP
                            sq = drain_and_store(
                                ps[:, mi * NSZ : (mi + 1) * NSZ], n, m, 1, sq
                            )
                    if not last:
                        sq = drain_and_store(ps, n, mp * 2, 2, sq)
    nc.compile()
    return nc


def _get_nc():
    if "nc" not in _NC_CACHE:
        _NC_CACHE["nc"] = _build_nc()
    return _NC_CACHE["nc"]


def _prep_in_maps(x, Wv):
    import ml_dtypes

    bf16 = ml_dtypes.bfloat16
    x = np.ascontiguousarray(np.asarray(x, dtype=np.float32))
    Wv = np.asarray(Wv, dtype=np.float32)

    xf = x.reshape(ROWS, E)
    # wv[p, (n*KT+k)*NSZ + j] = Wv[n*NSZ + j, k*P + p]
    wvp = np.ascontiguousarray(
        Wv.reshape(NT, NSZ, KT, P)
        .transpose(3, 0, 2, 1)
        .reshape(P, NT * KT * NSZ)
        .astype(bf16)
    )

    in_maps = []
    for c in range(N_CORES):
        xs = xf[c * R : (c + 1) * R]                    # [R, E]
        # xb[p, (m*KT+k)*P+mm] = xs[m*P+mm, k*P+p]
        xbc = np.ascontiguousarray(
            xs.reshape(MT, P, KT, P)
            .transpose(3, 0, 2, 1)
            .reshape(P, MT * KT * P)
            .astype(bf16)
        )
        in_maps.append({"xb": xbc, "wv": wvp})
    return in_maps


def _install_ntff_hook():
    """This image's antenv lacks axon_hooks; recreate the bridge module so
    run_bass_kernel_spmd(trace=True) can reach the ctypes NTFF profiler."""
    import types

    if "antenv.axon_hooks" in sys.modules:
        return
    try:
        from trn_agent_boot.trn_boot import _ntff_profile_via_ctypes
    except ImportError:
        return
    hook = _ntff_profile_via_ctypes("/opt/axon/libaxon_pjrt.so")
    mod = types.ModuleType("antenv.axon_hooks")
    mod._hook = hook
    mod.get_axon_ntff_profile_hook = lambda: mod._hook
    mod.set_axon_ntff_profile_hook = lambda h: setattr(mod, "_hook", h)
    sys.modules["antenv.axon_hooks"] = mod


def _run(x, Wv, bv, trace=False):
    from concourse.bass_utils import run_bass_kernel_spmd

    if trace:
        _install_ntff_hook()
    nc = _get_nc()
    in_maps = _prep_in_maps(x, Wv)
    res = run_bass_kernel_spmd(
        nc, in_maps, core_ids=list(range(N_CORES)), trace=trace
    )
    out = np.concatenate(
        [np.asarray(res.results[c]["out"]) for c in range(N_CORES)], axis=0
    )
    out = out.astype(np.float32) + np.asarray(bv, dtype=np.float32)[None, :]
    return out.reshape(B, S, E), res


def kernel(x, Wq, bq, Wk, bk, Wv, bv, weights):
    out, _ = _run(x, Wv, bv, trace=False)
    return out


def kernel_traced(x, Wq, bq, Wk, bk, Wv, bv, weights):
    """Like kernel() but with NTFF profiling; returns (out, BassKernelResults)."""
    out, res = _run(x, Wv, bv, trace=True)
    return out, res
